# revision 20
# baseline (speedup 1.0000x reference)
"""GraphSAGE-mean 2-layer GNN on 8 Trainium2 NeuronCores (Bass/Tile).

Sharding: nodes split into 8 contiguous ranges (rows c*12500..): core c
computes output rows for its range.  The full feature table is replicated per
core; layer-1 results are AllGather'd to rebuild the replicated table for
layer 2.

Aggregation: per core, edges (grouped by dst) are split into 4 passes by src
chunk of 32768 rows so src indices fit the int16 index format of the custom
dma_gather ucode (the ucode sign-extends indices, so 32767 is a hard limit).
Segment-sum runs on the tensor engine: for each 128-edge block a selection
matrix
  sel[e, m] = (dstl[e] == m) * invdeg[dst[e]]
is built in one fused DVE op from a constant iota tile, and
  psum[f, m] += msgs[e, f]^T @ sel[e, m]
accumulates weighted neighbor sums for one 128-node tile, feature-major.

Edges are emitted supertile-major (ST=4 destination tiles x all 4 src
passes): each tile's blocks accumulate across all passes in a single
dedicated PSUM bank (4 concurrent accumulators = 4 banks, one zero-region
each), then one ScalarE copy evacuates psum -> aggT(bf16).  This removes
the per-pass DVE adds of a pass-major schedule and shortens the
gather->matmul dependency chains.  Gather instructions never cross a
(supertile, pass) boundary.

Precision: the whole edge path (feature table, gathered messages, selection
matrices, matmul weight operands, the AllGather payload) runs in bf16 — this
enables the PE fast-weight-load path (4x faster LDWEIGHTS), doubles DVE
throughput on the selection builds, and halves collective traffic.  All
matmul accumulation stays fp32 in PSUM and the bias+relu epilogue is fp32,
so the end-to-end error stays ~1e-3 against the fp32 reference.

The self path: layer 1 loads x rows (512-row batched DMAs) and PE-transposes
them to feature-major; layer 2 reuses layer 1's feature-major post-relu
output tiles (oT) which are kept resident in SBUF — no reload / re-transpose.
PSUM->SBUF evacuations run on the Scalar engine to keep DVE free.

The SPMD program is shared by all 8 cores, so per-(pass, tile) block counts
are static = max over the 8 cores; shorter cores pad with zero-weight slots.
"""

import numpy as np

N = 100000
F = 128
NCORES = 8
OWN = N // NCORES            # 12500
P = 128
NTILES = (OWN + P - 1) // P  # 98
OWN_PAD = NTILES * P         # 12544
N_PAD = 100096               # table rows padded to a multiple of 128
CHUNK = 32768
NPASS = (N + CHUNK - 1) // CHUNK  # 4
GBS = 1024                   # gather rows per dma_gather instruction (SWDGE ring holds 1024 descs)
BLK = 128                    # edges per block
GPOOL_BUFS = 10              # gather tile double-buffer depth
SPOOL_BUFS = 10              # sel tile depth
PSA_BUFS = 2                 # (unused; acc banks are fixed at ST)
ST = 4                       # tiles per supertile (= concurrent PSUM acc banks)
NST = (NTILES + ST - 1) // ST


# --------------------------------------------------------------------------
# host-side planning
# --------------------------------------------------------------------------

def _plan(edge_src, edge_dst):
    src = np.asarray(edge_src).astype(np.int64).ravel()
    dst = np.asarray(edge_dst).astype(np.int64).ravel()
    deg = np.bincount(dst, minlength=N)
    invdeg = (1.0 / np.maximum(deg, 1)).astype(np.float32)

    per_core = []
    owner = dst // OWN
    for c in range(NCORES):
        m = owner == c
        s, d = src[m], dst[m]
        p = s // CHUNK
        stk = ((d - c * OWN) // P) // ST
        order = np.lexsort((d, p, stk))
        per_core.append((s[order], d[order], p[order]))

    cnt = np.zeros((NCORES, NPASS, NTILES), dtype=np.int64)
    for c in range(NCORES):
        s, d, p = per_core[c]
        t = (d - c * OWN) // P
        np.add.at(cnt, (c, p, t), 1)
    B = np.ceil(cnt.max(axis=0) / BLK).astype(np.int64)   # [NPASS, NTILES]

    # emission order: supertile-major, then pass, then tile.  Each tile's
    # blocks (across all passes of its supertile) accumulate in one PSUM
    # bank; blk_first/blk_last mark the accumulation group boundaries.
    cells = []
    for st in range(NST):
        t0, t1 = st * ST, min((st + 1) * ST, NTILES)
        for pp in range(NPASS):
            for t in range(t0, t1):
                if B[pp, t]:
                    cells.append((st, pp, t))
    blk_tile = np.concatenate(
        [np.full(B[pp, t], t, dtype=np.int64) for _st, pp, t in cells])
    nblk = int(blk_tile.shape[0])
    blk_first = np.zeros(nblk, dtype=bool)
    blk_last = np.zeros(nblk, dtype=bool)
    seen = set()
    for i, t in enumerate(blk_tile):
        if t not in seen:
            blk_first[i] = True
            seen.add(t)
    seen = set()
    for i in range(nblk - 1, -1, -1):
        t = int(blk_tile[i])
        if t not in seen:
            blk_last[i] = True
            seen.add(t)

    plans = []
    for c in range(NCORES):
        s, d, p = per_core[c]
        idx16 = np.zeros(nblk * BLK, dtype=np.int16)
        dstl = np.full(nblk * BLK, -1.0, dtype=np.float32)
        w = np.zeros(nblk * BLK, dtype=np.float32)
        cursor = 0
        blk0 = 0
        for _st, pp, t in cells:
            bcount = int(B[pp, t])
            ne = int(cnt[c, pp, t])
            se = s[cursor : cursor + ne]
            de = d[cursor : cursor + ne]
            assert ne <= bcount * BLK
            base = blk0 * BLK
            idx16[base : base + ne] = (se - pp * CHUNK).astype(np.int16)
            dstl[base : base + ne] = (de - c * OWN - t * P).astype(np.float32)
            w[base : base + ne] = invdeg[de]
            cursor += ne
            blk0 += bcount
        assert cursor == s.shape[0] and blk0 == nblk
        plans.append({"idx16": idx16, "dstl": dstl, "w": w})

    return plans, B, blk_tile, blk_first, blk_last, nblk


def _gather_instruction_sizes(B):
    """Mirror of the device loop: list of (pass, blocks) per gather inst.
    Gather instructions never cross a (supertile, pass) boundary."""
    out = []
    for st in range(NST):
        t0, t1 = st * ST, min((st + 1) * ST, NTILES)
        for pp in range(NPASS):
            nb = int(B[pp, t0:t1].sum())
            while nb > 0:
                take = min(GBS // BLK, nb)
                out.append((pp, take))
                nb -= take
    return out


def _pack_gidx(idx16, B):
    """Pack int16 indices in the dma_gather SBUF layout (position j ->
    partition j%16, column j//16, replicated to 128 partitions) as one
    [128, total_cols] plane with per-instruction column segments, raveled
    partition-major.  Loaded to SBUF once and sliced per instruction."""
    total_cols = sum(take * BLK // 16
                     for _pp, take in _gather_instruction_sizes(B))
    out = np.zeros((128, total_cols), dtype=np.int16)
    cursor = 0
    col = 0
    for _pp, take in _gather_instruction_sizes(B):
        rows = take * BLK
        seg = idx16[cursor : cursor + rows]
        cursor += rows
        w16 = seg.reshape(rows // 16, 16).T          # [16, cols]
        out[:, col : col + rows // 16] = np.tile(w16, (8, 1))
        col += rows // 16
    return out.ravel()


# --------------------------------------------------------------------------
# device program
# --------------------------------------------------------------------------

def _build(B, blk_tile, blk_first, blk_last, nblk, skip_collective=False,
           skip_gather=False, skip_edges=False, skip_self=False,
           skip_xform=False, nqueues=4):
    import concourse.bass as bass
    import concourse.mybir as mybir
    import concourse.tile as tile
    from concourse import library_config
    from concourse.masks import make_identity
    from concourse.tile_rust import add_dep_helper

    nc = bass.Bass("TRN2", target_bir_lowering=False, debug=False,
                   num_devices=NCORES, num_swdge_queues=4)
    dt = mybir.dt
    bf = dt.bfloat16

    x_rep = nc.dram_tensor("x_rep", [N_PAD, F], bf, kind="ExternalInput")
    x_self = nc.dram_tensor("x_self", [OWN_PAD, F], bf, kind="ExternalInput")
    gidx_len = sum(128 * (take * BLK // 16)
                   for _pp, take in _gather_instruction_sizes(B))
    gidx = nc.dram_tensor("gidx", [gidx_len], dt.int16, kind="ExternalInput")
    dstl_in = nc.dram_tensor("dstl", [P * nblk], dt.float32, kind="ExternalInput")
    w_in = nc.dram_tensor("w", [P * nblk], dt.float32, kind="ExternalInput")
    iota_in = nc.dram_tensor("iota", [P * P], bf, kind="ExternalInput")
    ws1 = nc.dram_tensor("W_self1", [F, F], bf, kind="ExternalInput")
    wn1 = nc.dram_tensor("W_neigh1", [F, F], bf, kind="ExternalInput")
    b1 = nc.dram_tensor("b1", [F], dt.float32, kind="ExternalInput")
    ws2 = nc.dram_tensor("W_self2", [F, F], bf, kind="ExternalInput")
    wn2 = nc.dram_tensor("W_neigh2", [F, F], bf, kind="ExternalInput")
    b2 = nc.dram_tensor("b2", [F], dt.float32, kind="ExternalInput")
    out_shard = nc.dram_tensor("out_shard", [OWN_PAD, F], dt.float32,
                               kind="ExternalOutput")

    h1_own = nc.dram_tensor("h1_own", [OWN_PAD, F], bf)
    h1_rep = nc.dram_tensor("h1_rep", [N_PAD, F], bf, addr_space="Shared")

    pass_len = [min(CHUNK, N - p * CHUNK) for p in range(NPASS)]
    inst_sizes = _gather_instruction_sizes(B)

    with tile.TileContext(nc) as tc:
        with (
            tc.tile_pool(name="const", bufs=1) as cpool,
            tc.tile_pool(name="gather", bufs=GPOOL_BUFS) as gpool,
            tc.tile_pool(name="sel", bufs=SPOOL_BUFS) as spool,
            tc.tile_pool(name="acc", bufs=1) as apool,
            tc.tile_pool(name="stage", bufs=3) as stpool,
            tc.tile_pool(name="psA", bufs=1, space="PSUM") as ppoolA,
            tc.tile_pool(name="psB", bufs=2, space="PSUM") as ppoolB,
        ):
            lib = nc.gpsimd.load_library(library_config.mlp)
            rows_regs = {}

            def rows_reg(v):
                if v not in rows_regs:
                    rows_regs[v] = nc.gpsimd.to_reg(v)
                return rows_regs[v]

            iota = cpool.tile([P, P], bf)
            nc.sync.dma_start(out=iota[:],
                              in_=iota_in.ap().rearrange("(p f) -> p f", p=P))
            ident = cpool.tile([P, P], dt.float32)
            make_identity(nc, ident[:])
            ident_bf = cpool.tile([P, P], bf)
            nc.vector.tensor_copy(out=ident_bf[:], in_=ident[:])

            wtiles = {}
            for name, t in (("ws1", ws1), ("wn1", wn1), ("ws2", ws2),
                            ("wn2", wn2)):
                wt = cpool.tile([P, P], bf, name=f"w_{name}", tag=f"w_{name}")
                nc.sync.dma_start(out=wt[:], in_=t[:, :])
                wtiles[name] = wt
            btiles = {}
            for name, t in (("b1", b1), ("b2", b2)):
                bt = cpool.tile([P, 1], dt.float32, name=f"b_{name}", tag=f"b_{name}")
                nc.sync.dma_start(out=bt[:], in_=t.ap()[:, None])
                btiles[name] = bt

            gidx_t = cpool.tile([P, gidx_len // P], dt.int16)
            nc.sync.dma_start(out=gidx_t[:],
                              in_=gidx.ap().rearrange("(p k) -> p k", p=P))
            dstl_t = cpool.tile([P, nblk], dt.float32)
            nc.sync.dma_start(out=dstl_t[:],
                              in_=dstl_in.ap().rearrange("(p b) -> p b", p=P))
            w_t = cpool.tile([P, nblk], dt.float32)
            nc.sync.dma_start(out=w_t[:],
                              in_=w_in.ap().rearrange("(p b) -> p b", p=P))

            aggT = apool.tile([P, NTILES * P], bf)
            selfT1 = apool.tile([P, NTILES * P], bf)
            selfT2 = apool.tile([P, NTILES * P], bf)

            def run_layer(table, wself, wneigh, bias, dest, ddt,
                          selfT, selfT_next):
                """table: bf16 DRAM gather source.  selfT: bf16 [P, OWN_PAD]
                feature-major self features (filled by caller).  dest: DRAM
                output, node-major, dtype ddt.  selfT_next: if not None,
                also emit the feature-major post-relu output (bf16) there."""
                nc.vector.memset(aggT[:], 0.0)

                acc = {}            # supertile-slot -> live psum bank tile
                blk_cursor = 0      # global block index
                gcol = 0            # idx columns consumed in gidx_t
                ginst = 0           # gather instruction counter
                for pp, take in inst_sizes:
                    rows = take * BLK
                    icols = rows // 16
                    gt = (None if skip_gather else
                          gpool.tile([P, (GBS // BLK) * P], bf, tag="g"))
                    if not skip_gather:
                        g = nc.gpsimd.dma_gather(
                            gt[:, : take * P].rearrange("p (b f) -> p b f", f=P),
                            table[pp * CHUNK : pp * CHUNK + pass_len[pp], :],
                            gidx_t[:, gcol : gcol + icols],
                            rows,
                            rows_reg(rows),
                            F,
                            queue_num=ginst % nqueues,
                        )
                        add_dep_helper(g.ins, lib.ins, sync=False,
                                       reason="ucode lib before gather")
                    gcol += icols
                    ginst += 1

                    for k in range(take if not skip_edges else 0):
                        b = blk_cursor + k
                        t = int(blk_tile[b])
                        sel = spool.tile([P, P], bf, tag="sel")
                        nc.vector.tensor_scalar(
                            sel[:], iota[:],
                            dstl_t[:, b : b + 1], w_t[:, b : b + 1],
                            mybir.AluOpType.is_equal, mybir.AluOpType.mult,
                        )
                        slot = t % ST
                        if blk_first[b]:
                            acc[slot] = ppoolA.tile([P, P], dt.float32,
                                                    name="ps", tag=f"acc{slot}",
                                                    space="PSUM")
                        ps = acc[slot]
                        lhsT = (ident_bf[:] if skip_gather
                                else gt[:, k * P : (k + 1) * P])
                        nc.tensor.matmul(
                            out=ps[:], lhsT=lhsT,
                            rhs=sel[:],
                            start=bool(blk_first[b]),
                            stop=bool(blk_last[b]),
                        )
                        if blk_last[b]:
                            nc.scalar.copy(
                                out=aggT[:, t * P : (t + 1) * P], in_=ps[:])
                            del acc[slot]
                    blk_cursor += take

                writes = []
                for g0 in range(0, NTILES, 4):
                    tn = min(4, NTILES - g0)
                    wdt = tn * P
                    psT = ppoolB.tile([P, 512], dt.float32, tag="psT",
                                      space="PSUM")
                    nc.tensor.matmul(out=psT[:, :wdt], lhsT=wneigh[:],
                                     rhs=aggT[:, g0 * P : g0 * P + wdt],
                                     start=True, stop=False)
                    nc.tensor.matmul(out=psT[:, :wdt], lhsT=wself[:],
                                     rhs=selfT[:, g0 * P : g0 * P + wdt],
                                     start=False, stop=True)
                    oT = stpool.tile([P, 512], dt.float32, tag="oT")
                    nc.scalar.activation(oT[:, :wdt], psT[:, :wdt],
                                         mybir.ActivationFunctionType.Relu,
                                         bias=bias[:, :1])
                    if selfT_next is not None:
                        nc.vector.tensor_copy(
                            out=selfT_next[:, g0 * P : g0 * P + wdt],
                            in_=oT[:, :wdt])
                    ost = stpool.tile([P, 512], ddt, tag="ost")
                    for j in range(tn):
                        psX = ppoolA.tile([P, P], dt.float32, tag="psX",
                                          space="PSUM")
                        nc.tensor.transpose(out=psX[:],
                                            in_=oT[:, j * P : (j + 1) * P],
                                            identity=ident[:])
                        nc.scalar.copy(
                            out=ost[:, j * P : (j + 1) * P], in_=psX[:])
                    dd = nc.sync.dma_start(
                        out=dest[g0 * P : g0 * P + wdt, :]
                        .rearrange("(j p) f -> p j f", p=P),
                        in_=ost[:, :wdt].rearrange("p (j f) -> p j f", f=P),
                    )
                    writes.append(dd)
                return writes

            # layer-1 self path: batched loads of x_self + PE transpose to
            # feature-major bf16 selfT1.
            for g0 in range(0, NTILES if not skip_self else 0, 4):
                tn = min(4, NTILES - g0)
                wdt = tn * P
                xtw = stpool.tile([P, 512], bf, tag="xtw")
                nc.sync.dma_start(
                    out=xtw[:, :wdt].rearrange("p (j f) -> p j f", f=F),
                    in_=x_self[g0 * P : g0 * P + wdt, :]
                    .rearrange("(j p) f -> p j f", p=P))
                for j in range(tn):
                    pst = ppoolA.tile([P, P], bf, tag="pst",
                                      space="PSUM")
                    nc.tensor.transpose(out=pst[:],
                                        in_=xtw[:, j * P : (j + 1) * P],
                                        identity=ident_bf[:])
                    nc.scalar.copy(
                        out=selfT1[:, (g0 + j) * P : (g0 + j + 1) * P],
                        in_=pst[:])
            if skip_self:
                nc.vector.memset(selfT1[:], 0.0)

            if skip_xform:
                zt0 = stpool.tile([P, F], dt.float32, tag="zx")
                nc.vector.memset(zt0[:], 0.0)
                ztb = stpool.tile([P, F], mybir.dt.bfloat16, tag="zxb")
                nc.vector.memset(ztb[:], 0.0)
                for g0 in range(0, NTILES):
                    nc.sync.dma_start(out=h1_own[g0 * P : (g0 + 1) * P, :],
                                      in_=ztb[:])
                    nc.sync.dma_start(out=out_shard[g0 * P : (g0 + 1) * P, :],
                                      in_=zt0[:])
                nc.vector.memset(selfT2[:], 0.0)
            else:
                run_layer(x_rep, wtiles["ws1"], wtiles["wn1"], btiles["b1"],
                          h1_own, bf, selfT1, selfT2)

            if skip_collective:
                nc.sync.dma_start(out=h1_rep[0:OWN, :], in_=h1_own[0:OWN, :])
            else:
                nc.gpsimd.collective_compute(
                    "AllGather",
                    mybir.AluOpType.bypass,
                    replica_groups=[list(range(NCORES))],
                    ins=[h1_own[0:OWN, :]],
                    outs=[h1_rep[0:N, :]],
                )
            if N_PAD > N:
                zt = stpool.tile([P, F], bf, tag="zt")
                nc.vector.memset(zt[:], 0.0)
                nc.sync.dma_start(out=h1_rep[N:N_PAD, :],
                                  in_=zt[: N_PAD - N, :])

            if not skip_xform:
                run_layer(h1_rep, wtiles["ws2"], wtiles["wn2"], btiles["b2"],
                          out_shard, mybir.dt.float32, selfT2, None)

    _split_multi_waits(nc)
    from concourse.library_overlay import lower_extended_insts
    lower_extended_insts(nc)
    return nc


def _split_multi_waits(nc):
    """Walrus codegen encodes at most one sync wait per instruction; split
    extras into standalone EventSemaphore instructions on the same in-order
    engine queue (semantically identical)."""
    import concourse.mybir as mybir

    n = 0
    for f in nc.m.functions:
        for b in f.blocks:
            insts = b.instructions
            new_list = []
            for inst in insts:
                si = inst.sync_info
                if si is not None and len(si.on_wait) > 1:
                    waits = list(si.on_wait)
                    for wt in waits[:-1]:
                        ev = mybir.InstEventSemaphore(
                            name=f"evsplit-{n}",
                            engine=inst.engine,
                            sync_info=mybir.SyncInfo(on_wait=[wt],
                                                     on_update=[]),
                            ins=[], outs=[],
                        )
                        new_list.append(ev)
                        try:
                            nc.inst_map[ev.name] = ev
                        except Exception:
                            pass
                        n += 1
                    inst.sync_info = mybir.SyncInfo(
                        on_wait=[waits[-1]], on_update=list(si.on_update)
                    )
                new_list.append(inst)
            insts[:] = new_list
    return n


# --------------------------------------------------------------------------
# entry point
# --------------------------------------------------------------------------

def _prepare(x, edge_src, edge_dst, W_self1, W_neigh1, b1, W_self2, W_neigh2,
             b2):
    """Host-side planning + program build; returns (nc, in_maps)."""
    import ml_dtypes
    bf16 = ml_dtypes.bfloat16

    x = np.asarray(x, dtype=np.float32)
    plans, B, blk_tile, blk_first, blk_last, nblk = _plan(edge_src, edge_dst)

    xpad = np.zeros((N_PAD, F), dtype=bf16)
    xpad[:N] = x.astype(bf16)
    iota = np.broadcast_to(np.arange(P, dtype=np.float32),
                           (P, P)).astype(bf16)

    in_maps = []
    for c in range(NCORES):
        pl = plans[c]
        xs = np.zeros((OWN_PAD, F), dtype=bf16)
        xs[:OWN] = x[c * OWN : (c + 1) * OWN].astype(bf16)
        in_maps.append({
            "x_rep": xpad,
            "x_self": xs,
            "gidx": _pack_gidx(pl["idx16"], B),
            "dstl": pl["dstl"].reshape(nblk, P).T.copy().ravel(),
            "w": pl["w"].reshape(nblk, P).T.copy().ravel(),
            "iota": np.ascontiguousarray(iota).ravel(),
            "W_self1": np.asarray(W_self1, np.float32).astype(bf16),
            "W_neigh1": np.asarray(W_neigh1, np.float32).astype(bf16),
            "b1": np.asarray(b1, np.float32),
            "W_self2": np.asarray(W_self2, np.float32).astype(bf16),
            "W_neigh2": np.asarray(W_neigh2, np.float32).astype(bf16),
            "b2": np.asarray(b2, np.float32),
        })

    nc = _build(B, blk_tile, blk_first, blk_last, nblk)
    return nc, in_maps


def kernel(x, edge_src, edge_dst, W_self1, W_neigh1, b1, W_self2, W_neigh2,
           b2, trace=False, _return_res=False):
    from concourse.bass_utils import run_bass_kernel_spmd

    nc, in_maps = _prepare(x, edge_src, edge_dst, W_self1, W_neigh1, b1,
                           W_self2, W_neigh2, b2)
    res = run_bass_kernel_spmd(nc, in_maps, list(range(NCORES)), trace=trace)
    out = np.concatenate(
        [res.results[c]["out_shard"][:OWN] for c in range(NCORES)], axis=0
    ).astype(np.float32)
    if _return_res:
        return out, res
    return out
